# revision 56
# baseline (speedup 1.0000x reference)
"""Llama GQA attention (B=2, S=2048, D=2048, NH=16, NKV=4, HD=128) on 8 TRN2 cores.

Sharding: core c = batch*4 + kv_group  (2 batches x 4 kv groups).
Each core computes 4 q heads + 1 kv head for one batch, then a partial
output projection with its 512-row slice of Wo; the host sums the 4
partials per batch.

Device-side layout trick: everything is computed in "transposed"
orientation (QT/KT = [head_dim, S]) so that
  - projections take x^T tiles as the moving operand (host supplies x^T),
  - scoresT[sk, sq] = KT_tile^T @ QT needs no transposes,
  - softmax exp runs on scoresT, and
  - attn_outT[d, sq] accumulates with lhsT = V tile in natural [s, d]
    layout, rhs = expT; attn_outT then feeds the Wo matmul as lhsT.
Causal structure: score/AV matmuls are only emitted for lower-triangular
(sk, sq) tile pairs (diagonal-first within each q s-tile group, so the
diagonal exp has a full pipeline LAG of slack).

Softmax denominators: exp tiles are summed across sk on the DVE (fp16,
2x mode) into one [128, 512] accumulator per q s-tile; a gpsimd
partition_all_reduce (broadcast output) + DVE reciprocal then finish the
normalization entirely off the PE. This removes the per-pair denominator
matmuls of the v1 kernel (the single largest block of non-GEMM PE time).
Causal masking of the diagonal blocks also rides the PE's score PSUM
accumulation (negtri^T @ irep adds -60000 above the diagonal, exp
underflows to exact 0), so no mask op sits between exp and AV.

RoPE: host permutes Wq/Wk columns within each head to evens-then-odds, so
the interleaved-pair rotation becomes contiguous half-rotations on the
partition dim. Per projection: one DVE copy PSUM->fp16 SBUF (frees the
PSUM bank immediately so the score pipeline never waits on rope), then
four half-width fp16 2x-mode multiplies against [cos;sin] and [sin;cos]
tables plus one sub and one add (per-product operands share a base
partition - the BIR verifier requires it for SBUF/SBUF inputs).

Everything on-device is fp16 (not bf16): identical speed on every engine
but ~8x less quantization error, which pays for the fp16 denominator
accumulation. Softmax max-subtraction is skipped: scores/sqrt(128) are
O(10), exp is safe, and fp16's 65504 range covers the denominators.
"""

import os
import sys

if "/opt/trn_rl_repo" not in sys.path:
    sys.path.insert(0, "/opt/trn_rl_repo")

from contextlib import ExitStack

import numpy as np

import concourse.bass as bass
import concourse.tile as tile
from concourse import bacc, mybir
from concourse import bass_utils

B, S, D = 2, 2048, 2048
NH, NKV, HD = 16, 4, 128
G = NH // NKV  # q heads per core
N_CORES = 8
CH = 512  # sq chunk width
NCH = S // CH  # 4
DT = D // 128  # 16 d-tiles
ST = S // 128  # 16 s-tiles
SCALE = 1.0 / float(np.sqrt(HD))
ROPE_BASE = 10000.0
F16 = mybir.dt.float16
F32 = mybir.dt.float32


def build_kernel():
    nc = bacc.Bacc("TRN2", target_bir_lowering=False, debug=False, num_devices=N_CORES)
    xT = nc.dram_tensor("xT", [D, S], F16, kind="ExternalInput").ap()
    wq = nc.dram_tensor("wq", [D, G * HD], F16, kind="ExternalInput").ap()
    wk = nc.dram_tensor("wk", [D, HD], F16, kind="ExternalInput").ap()
    wv = nc.dram_tensor("wv", [D, HD], F16, kind="ExternalInput").ap()
    wo = nc.dram_tensor("wo", [G * HD, D], F16, kind="ExternalInput").ap()
    csA = nc.dram_tensor("csA", [128, S], F16, kind="ExternalInput").ap()
    csB = nc.dram_tensor("csB", [128, S], F16, kind="ExternalInput").ap()
    negtri = nc.dram_tensor("negtri", [128, 128], F16, kind="ExternalInput").ap()
    irep = nc.dram_tensor("irep", [128, G * 128], F16, kind="ExternalInput").ap()
    out = nc.dram_tensor("out", [S, D], F16, kind="ExternalOutput").ap()

    with tile.TileContext(nc) as tc, ExitStack() as ctx:
        consts = ctx.enter_context(tc.tile_pool(name="consts", bufs=1))
        work = ctx.enter_context(tc.tile_pool(name="work", bufs=1))
        ps = ctx.enter_context(tc.tile_pool(name="ps", bufs=8, space="PSUM"))

        wq_s = consts.tile([128, DT, G * HD], F16)
        wk_s = consts.tile([128, DT, HD], F16)
        wv_s = consts.tile([128, DT, HD], F16)
        wo_s = consts.tile([128, G, D], F16)
        csA_s = consts.tile([128, S], F16)  # cos rows 0-63, sin rows 64-127
        csB_s = consts.tile([128, S], F16)  # sin rows 0-63, cos rows 64-127
        negtri_s = consts.tile([128, 128], F16)  # [p, sk] = -60000 if sk > p
        irep_s = consts.tile([128, G * 128], F16)  # eye(128) tiled G times
        ones_s = consts.tile([128, 1], F16)
        warm_s = consts.tile([1, 1], F32)

        qt = work.tile([128, G, S], F16)  # Q^T per head, rope'd
        kt = work.tile([128, S], F16)  # K^T, rope'd
        v = work.tile([128, ST, HD], F16)  # V natural [s, d] tiles
        att = work.tile([128, G, S], F16)  # attn_out^T per head

        nc.vector.memset(ones_s[:], 1.0)
        nc.vector.memset(warm_s[:], 0.0)
        # pull the Exp act table load off the critical path: attach it to a
        # dummy activation that runs during the startup DMAs
        nc.scalar.activation(
            warm_s[:], warm_s[:], mybir.ActivationFunctionType.Exp, scale=1.0
        )

        # ---- fused per-chunk pipeline ----
        # For each 512-wide sq chunk c: stream x^T chunk, project q/k/v for
        # that chunk (one rolling PSUM bank per r-block), copy+rope the
        # projections, run the chunk's attention (sk-pipelined scores+exp vs
        # AV matmuls + DVE denominator accumulation), normalize, and do the
        # chunk's slice of the output projection. Later chunks' DMA /
        # projection overlaps earlier chunks' attention.
        xsp = ctx.enter_context(tc.tile_pool(name="xsp", bufs=3))
        ropet = ctx.enter_context(tc.tile_pool(name="ropet", bufs=3))
        ehp = ctx.enter_context(tc.tile_pool(name="ehp", bufs=12))
        dsp = ctx.enter_context(tc.tile_pool(name="dsp", bufs=3))
        nrm = ctx.enter_context(tc.tile_pool(name="nrm", bufs=int(os.environ.get("K_NRM", "3"))))
        otp = ctx.enter_context(tc.tile_pool(name="otp", bufs=3))

        xT_r = xT.rearrange("(dt p) s -> p dt s", p=128)
        wq_r = wq.rearrange("(dt p) n -> p dt n", p=128)
        wk_r = wk.rearrange("(dt p) n -> p dt n", p=128)
        wv_r = wv.rearrange("(dt p) n -> p dt n", p=128)
        wo_r = wo.rearrange("(ci p) n -> p ci n", p=128)

        def rope_block(r, pst, c):
            # pst: PSUM [128, CH] f32; evens on partitions 0-63, odds 64-127.
            # One DVE copy frees the PSUM bank, then 4 fp16 2x-mode DVE ops:
            #   mulA = [qe*cos ; qo*sin], mulB = [qe*sin ; qo*cos]
            #   even out = mulA[0:64] - mulA[64:128]
            #   odd  out = mulB[0:64] + mulB[64:128]
            csl = slice(c * CH, (c + 1) * CH)
            if r < G:
                dest_even = qt[0:64, r, csl]
                dest_odd = qt[64:128, r, csl]
            else:
                dest_even = kt[0:64, csl]
                dest_odd = kt[64:128, csl]
            # NB: when both DVE inputs are SBUF the verifier requires equal
            # base partitions, so each product pairs operands from the same
            # 64-partition half; only outputs may move across the base.
            pb = ropet.tile([128, CH], F16, tag="pb")
            nc.vector.tensor_copy(pb[:], pst)
            t1 = ropet.tile([64, CH], F16, tag="t1")
            nc.vector.tensor_mul(t1[:], pb[0:64, :], csA_s[0:64, csl])
            t2 = ropet.tile([64, CH], F16, tag="t2")
            nc.vector.tensor_mul(t2[:], pb[64:128, :], csA_s[64:128, csl])
            t3 = ropet.tile([64, CH], F16, tag="t3")
            nc.vector.tensor_mul(t3[:], pb[0:64, :], csB_s[0:64, csl])
            t4 = ropet.tile([64, CH], F16, tag="t4")
            nc.vector.tensor_mul(t4[:], pb[64:128, :], csB_s[64:128, csl])
            nc.vector.tensor_sub(dest_even, t1[:], t2[:])
            nc.vector.tensor_add(dest_odd, t3[:], t4[:])

        LAG = int(os.environ.get("K_LAG", "10"))

        xs_tiles = {}

        def emit_cs(c):
            csl = slice(c * CH, (c + 1) * CH)
            nc.sync.dma_start(csA_s[:, csl], csA[:, csl])
            nc.sync.dma_start(csB_s[:, csl], csB[:, csl])

        def emit_xs(c):
            if c == 1 and 1 in xs_tiles:
                return
            csl = slice(c * CH, (c + 1) * CH)
            xsc = xsp.tile([128, DT, CH], F16, tag="xs", name=f"xs_{c}")
            if c == 0:
                # all startup DMAs ride the SP queue in priority order (the
                # transfers serialize on the shared DMA engines anyway, and
                # the ACT queue must stay clear for exps). The k/q
                # projections chase the x/wq streams d-tile by d-tile; the
                # rope tables stream per chunk, just in time for each rope.
                # 4-d-tile granularity: a 512KB transfer (1.5us) outlasts the
                # per-copy HWDGE descriptor-gen (625ns), so the stream runs
                # at full DMA bandwidth instead of descriptor-bound ~55%
                csl1 = slice(CH, 2 * CH)
                xsc1 = xsp.tile([128, DT, CH], F16, tag="xs", name="xs_1")
                nc.sync.dma_start(wk_s[:, 0:4, :], wk_r[:, 0:4, :])
                nc.sync.dma_start(xsc[:, 0:4, :], xT_r[:, 0:4, csl])
                nc.sync.dma_start(wk_s[:, 4:16, :], wk_r[:, 4:16, :])
                nc.sync.dma_start(xsc[:, 4:8, :], xT_r[:, 4:8, csl])
                nc.sync.dma_start(wq_s[:, 0:4, :], wq_r[:, 0:4, :])
                nc.sync.dma_start(xsc[:, 8:12, :], xT_r[:, 8:12, csl])
                nc.sync.dma_start(wq_s[:, 4:8, :], wq_r[:, 4:8, :])
                nc.sync.dma_start(xsc[:, 12:16, :], xT_r[:, 12:16, csl])
                nc.sync.dma_start(wq_s[:, 8:16, :], wq_r[:, 8:16, :])
                nc.sync.dma_start(wv_s[:], wv_r[:])
                emit_cs(0)
                nc.sync.dma_start(negtri_s[:], negtri[:])
                nc.sync.dma_start(irep_s[:], irep[:])
                nc.sync.dma_start(xsc1[:, 0:8, :], xT_r[:, 0:8, csl1])
                nc.sync.dma_start(xsc1[:, 8:16, :], xT_r[:, 8:16, csl1])
                emit_cs(1)
                xs_tiles[1] = xsc1
            else:
                emit_cs(c)
                nc.sync.dma_start(xsc[:, 0:8, :], xT_r[:, 0:8, csl])
                nc.sync.dma_start(xsc[:, 8:16, :], xT_r[:, 8:16, csl])
            xs_tiles[c] = xsc

        def emit_wo_loads():
            # Wo row-blocks are first needed by the chunk-0 output
            # projections (~halfway through chunk-0 attention); queued after
            # the chunk-2 x prefetch so they never delay the projections.
            for ci in range(G):
                nc.sync.dma_start(wo_s[:, ci, :], wo_r[:, ci, :])

        def make_proj_block(c, r, xsc):
            # k/q projection for chunk c, head-slot r: a PE part (16
            # accumulating matmuls) and a finisher (PSUM->SBUF copy + rope on
            # the DVE), which the scheduler lags a couple of pairs behind the
            # PE part so the in-order DVE never stalls on the matmuls.
            holder = {}

            def mm():
                if r < G:
                    w_view = wq_s[:, :, r * HD : (r + 1) * HD]
                else:
                    w_view = wk_s[:, :, :]
                pp = ps.tile([128, CH], F32, tag="ps", name=f"pp_{c}_{r}")
                warm = c == 0 and r == G
                for d in range(DT):
                    nc.tensor.matmul(
                        pp[:],
                        lhsT=w_view[:, d, :],
                        rhs=xsc[:, d, :],
                        start=(d == 0),
                        stop=(d == DT - 1),
                    )
                    if warm and d in (3, 7):
                        # k-projection chases the startup x stream; these
                        # dummies on already-arrived data keep the PE busy
                        # (and its p-state ramped) through the DMA wait
                        wp = ps.tile([128, CH], F32, tag="ps", name=f"wp_{d}")
                        for _ in range(2):
                            nc.tensor.matmul(
                                wp[:],
                                lhsT=w_view[:, 0, :],
                                rhs=xsc[:, 0, :],
                                start=True,
                                stop=True,
                            )
                holder["pp"] = pp

            def fin():
                rope_block(r, holder["pp"][:], c)

            return mm, fin

        def make_v_block(c, stl, xsc):
            # V in natural [s, d] layout directly: x^T tiles stationary,
            # wv moving -- no V^T intermediate, no PE transposes
            holder = {}
            st = 4 * c + stl

            def mm():
                vp = ps.tile([128, 128], F32, tag="ps", name=f"vp_{st}")
                for d in range(DT):
                    nc.tensor.matmul(
                        vp[:],
                        lhsT=xsc[:, d, stl * 128 : (stl + 1) * 128],
                        rhs=wv_s[:, d, :],
                        start=(d == 0),
                        stop=(d == DT - 1),
                    )
                holder["vp"] = vp

            def fin():
                nc.vector.tensor_copy(v[:, st, :], holder["vp"][:])

            return mm, fin

        def chunk_blocks(c):
            xsc = xs_tiles.pop(c)
            return [make_proj_block(c, r, xsc) for r in [G] + list(range(G))] + [
                make_v_block(c, stl, xsc) for stl in range(4)
            ]

        def emit_proj(c):
            # startup (non-interleaved) form: software-pipeline each block's
            # finisher behind the next block's matmuls
            prev_fin = None
            for mm, fin in chunk_blocks(c):
                mm()
                if prev_fin is not None:
                    prev_fin()
                prev_fin = fin
            prev_fin()

        # ---- attention + output projection over 128-wide sq tiles ----
        # For each (st, sk) causal pair: ONE score matmul (4 heads packed on
        # the moving side, N=512), one exp, one AV matmul (all heads into one
        # PSUM bank), and a DVE add into the s-tile's denominator
        # accumulator. av lags scores by LAG pairs so the PE never waits on
        # the exp of its own pair. After an s-tile's last AV, a single
        # ones^T @ dsum matmul yields the denominators, then the normalize +
        # Wo output projection are emitted (deferred 2 tiles).
        state = {"avp": None, "pending": None, "avctr": 0, "wo_pace": int(os.environ.get("K_PACE", "2"))}
        ehs = {}
        raw_eh = {}  # st -> first exp tile of the group (pre-accumulation)
        ds_acc = {}  # st -> fp16 denominator accumulator

        def emit_sc(st, sk, first, last):
            stsl = slice(st * 128, (st + 1) * 128)
            scp = ps.tile([128, CH], F32, tag="ps", name=f"sc_{st}_{sk}")
            diag = sk == st
            nc.tensor.matmul(
                scp[:],
                lhsT=kt[:, sk * 128 : (sk + 1) * 128],
                rhs=qt[:, :, stsl],
                start=True,
                stop=not diag,
            )
            if diag:
                # causal masking in PSUM: adds -60000 to the upper triangle
                # (negtri^T @ irep replicates the triangle across heads), so
                # exp underflows to exactly 0 - no DVE op in the chain
                nc.tensor.matmul(
                    scp[:],
                    lhsT=negtri_s[:],
                    rhs=irep_s[:],
                    start=False,
                    stop=True,
                )
            eh = ehp.tile([128, G, 128], F16, tag="eh", name=f"eh_{st}_{sk}")
            nc.scalar.activation(
                eh[:].rearrange("p h s -> p (h s)"),
                scp[:],
                mybir.ActivationFunctionType.Exp,
                scale=SCALE,
            )
            ehs[(st, sk)] = eh
            if st == ST - 1:
                den_accum(st, eh, eh[:].rearrange("p h s -> p (h s)"), first)
                if last:
                    # the group's denominator is complete LAG pairs before its
                    # last AV: reduce + reciprocal now, off the tail
                    dnb = nrm.tile([128, CH], F32, tag="bc")
                    nc.gpsimd.partition_all_reduce(
                        dnb[:],
                        ds_acc[st][:],
                        channels=128,
                        reduce_op=bass_isa.ReduceOp.add,
                    )
                    rec = nrm.tile([128, CH], F32, tag="rec")
                    nc.vector.reciprocal(rec[:], dnb[:])
                    state["final_rec"] = rec

        def den_accum(st, eh, ehf, first):
            if first:
                raw_eh[st] = eh
            elif st in raw_eh:
                ds = dsp.tile([128, CH], F16, tag="ds", name=f"ds_{st}")
                nc.vector.tensor_add(
                    ds[:],
                    raw_eh.pop(st)[:].rearrange("p h s -> p (h s)"),
                    ehf,
                )
                ds_acc[st] = ds
            else:
                ds = ds_acc[st]
                nc.vector.tensor_add(ds[:], ds[:], ehf)

        def flush_den():
            # the s-tile's denominator matmul + normalize, deferred one pair
            # past its last AV so the PE never waits on the DVE add chain
            pend = state["pending"]
            if pend is None:
                return
            state["pending"] = None
            st, avp = pend
            if st == ST - 1:
                ds_acc.pop(st)
                emit_norm(st, avp, None)
            else:
                if st in raw_eh:  # single-pair group (st == 0)
                    drhs = raw_eh.pop(st)[:].rearrange("p h s -> p (h s)")
                else:
                    drhs = ds_acc.pop(st)[:]
                emit_norm(st, avp, drhs)
            wo_units.extend((st, n) for n in range(NCH))

        def emit_av(st, sk, first, last):
            flush_den()
            if first:
                state["avp"] = ps.tile([128, CH], F32, tag="ps", name=f"av_{st}")
            avp = state["avp"]
            eh = ehs.pop((st, sk))
            ehf = eh[:].rearrange("p h s -> p (h s)")
            # denominator accumulation on the DVE (fp16 2x mode), lagged to
            # AV-time so the exp it reads is guaranteed complete and the
            # in-order DVE never blocks on the ACT engine. The final s-tile
            # group accumulates eagerly at score-time instead (see emit_sc)
            # so its reduction chain finishes during the last AVs.
            if st != ST - 1:
                den_accum(st, eh, ehf, first)
            nc.tensor.matmul(
                avp[:],
                lhsT=v[:, sk, :],
                rhs=ehf,
                start=first,
                stop=last,
            )
            if last:
                state["pending"] = (st, avp)
            # fill the exp-wait gap after each AV with one 512-column slice
            # of a deferred output projection; paced so the fillers last the
            # whole chunk instead of front-loading
            state["avctr"] += 1
            if wo_units and state["avctr"] % state["wo_pace"] == 0:
                emit_wo_unit(*wo_units.pop(0))

        wo_units = []

        def emit_norm(st, avp, drhs):
            # the whole denominator reduction runs off the PE: gpsimd
            # all-reduce sums the fp16 accumulator across partitions (with
            # broadcast output), DVE takes the reciprocal and normalizes
            stsl = slice(st * 128, (st + 1) * 128)
            if st == ST - 1:
                rec = state.pop("final_rec")
            else:
                dnb = nrm.tile([128, CH], F32, tag="bc")
                nc.gpsimd.partition_all_reduce(
                    dnb[:], drhs, channels=128, reduce_op=bass_isa.ReduceOp.add
                )
                rec = nrm.tile([128, CH], F32, tag="rec")
                nc.vector.reciprocal(rec[:], dnb[:])
            if st == ST - 1:
                # final s-tile: per-head muls so the tail Wo matmuls can
                # start after head 0 instead of after the full normalize
                for h in range(G):
                    nc.vector.tensor_mul(
                        att[:, h, stsl],
                        avp[:, h * 128 : (h + 1) * 128],
                        rec[:, h * 128 : (h + 1) * 128],
                    )
            else:
                nc.vector.tensor_mul(
                    att[:, :, stsl],
                    avp[:].rearrange("p (h s) -> p h s", h=G),
                    rec[:].rearrange("p (h s) -> p h s", h=G),
                )

        ots = {}

        def emit_wo_unit(st, n):
            stsl = slice(st * 128, (st + 1) * 128)
            if n == 0:
                ots[st] = otp.tile([128, S], F16, tag="ot", name=f"ot_{st}")
            ot = ots[st]
            po = ps.tile([128, CH], F32, tag="ps", name=f"po_{st}_{n}")
            for ci in range(G):
                nc.tensor.matmul(
                    po[:],
                    lhsT=att[:, ci, stsl],
                    rhs=wo_s[:, ci, n * CH : (n + 1) * CH],
                    start=(ci == 0),
                    stop=(ci == G - 1),
                )
            # PSUM->SBUF copies split across ACT and DVE (GPSIMD has no
            # PSUM port): ACT is exp-bound in the late chunks, DVE is
            # rope-bound in the early ones
            if n % 2 == 0:
                nc.scalar.copy(ot[:, n * CH : (n + 1) * CH], po[:])
            else:
                nc.vector.tensor_copy(ot[:, n * CH : (n + 1) * CH], po[:])
            nc.sync.dma_start(
                out[st * 128 : (st + 1) * 128, n * CH : (n + 1) * CH],
                ot[:, n * CH : (n + 1) * CH],
            )
            if n == NCH - 1:
                ots.pop(st)

        emit_xs(0)
        emit_proj(0)
        emit_xs(1)
        emit_proj(1)
        emit_xs(2)
        emit_wo_loads()
        for c in range(NCH):
            # diagonal-first pair order: the diagonal exp (the one the
            # group's stop-AV waits on) is issued G pairs early, so it gets
            # the full LAG of slack instead of none
            pairs = []
            for st in range(4 * c, 4 * c + 4):
                sks = [st] + list(range(st))
                for j, sk in enumerate(sks):
                    pairs.append((st, sk, j == 0, j == len(sks) - 1))
            npair = len(pairs)
            # interleave chunk c+1's projection blocks into the pair stream:
            # the PE fills exp-wait gaps with projection matmuls while the
            # ACT engine drains, instead of a monolithic projection phase
            sched = [[] for _ in range(npair + 1)]
            if c + 2 < NCH:
                blocks = chunk_blocks(c + 2)
                nb = len(blocks)
                for j, (mm, fin) in enumerate(blocks):
                    i_mm = min(npair - 1, 1 + (j * npair) // nb)
                    i_fin = min(npair, i_mm + 2)
                    sched[i_mm].append(mm)
                    sched[i_fin].append(fin)
            if c == NCH - 1:
                state["wo_pace"] = int(os.environ.get("K_PACE3", "2"))
            lag_c = min(LAG + int(os.environ.get("K_LAGB", "0")) * (c >= 2), npair - 1)
            for i, p in enumerate(pairs):
                emit_sc(*p)
                if i >= lag_c:
                    emit_av(*pairs[i - lag_c])
                for fn in sched[i]:
                    fn()
            if c + 3 < NCH:
                emit_xs(c + 3)
            for p in pairs[npair - lag_c :]:
                emit_av(*p)
            for fn in sched[npair]:
                fn()
            flush_den()
            # drain deferred output projections down to one s-tile (kept to
            # hide the just-started normalize chain under the next chunk's
            # scores); the final chunk drains fully
            keep = 0 if c == NCH - 1 else NCH
            while len(wo_units) > keep:
                emit_wo_unit(*wo_units.pop(0))

    nc.compile()
    return nc


_EXEC_CACHE = None


def _get_exec():
    """Build the Bass program once and wrap it in a cached jitted shard_map.

    Inputs are concatenated on axis 0 across a (batch=2, group=4) device mesh
    so that batch-replicated tensors (weights, tables) and group-replicated
    tensors (x^T) are only transferred once each, and the jit/NEFF lowering
    happens a single time per process.
    """
    global _EXEC_CACHE
    if _EXEC_CACHE is not None:
        return _EXEC_CACHE

    import jax
    from jax.experimental.shard_map import shard_map
    from jax.sharding import Mesh, PartitionSpec

    from concourse import bass2jax, mybir as _mybir

    nc = build_kernel()
    bass2jax.install_neuronx_cc_hook()

    partition_name = (
        nc.partition_id_tensor.name if nc.partition_id_tensor is not None else None
    )
    in_names = []
    out_names = []
    out_avals = []
    for alloc in nc.m.functions[0].allocations:
        if not isinstance(alloc, _mybir.MemoryLocationSet):
            continue
        name = alloc.memorylocations[0].name
        if alloc.kind == "ExternalInput":
            if name != partition_name:
                in_names.append(name)
        elif alloc.kind == "ExternalOutput":
            out_names.append(name)
            out_avals.append(
                jax.core.ShapedArray(
                    tuple(alloc.tensor_shape), _mybir.dt.np(alloc.dtype)
                )
            )
    n_params = len(in_names)
    all_in_names = tuple(in_names) + tuple(out_names)
    if partition_name is not None:
        all_in_names = all_in_names + (partition_name,)

    def _body(*args):
        operands = list(args)
        if partition_name is not None:
            operands.append(bass2jax.partition_id_tensor())
        outs = bass2jax._bass_exec_p.bind(
            *operands,
            out_avals=tuple(out_avals),
            in_names=all_in_names,
            out_names=tuple(out_names),
            lowering_input_output_aliases=(),
            sim_require_finite=True,
            sim_require_nnan=True,
            nc=nc,
        )
        return tuple(outs)

    devices = jax.devices()[:N_CORES]
    mesh = Mesh(np.asarray(devices).reshape(B, NKV), ("b", "g"))
    # sharding of the axis-0-concatenated global inputs, in in_names order
    spec_by_name = {
        "xT": PartitionSpec("b"),  # [B*D, S]
        "wq": PartitionSpec("g"),  # [NKV*D, G*HD]
        "wk": PartitionSpec("g"),
        "wv": PartitionSpec("g"),
        "wo": PartitionSpec("g"),  # [NKV*G*HD, D]
        "csA": PartitionSpec(),  # replicated
        "csB": PartitionSpec(),
        "tri": PartitionSpec(),
        "out": PartitionSpec(("b", "g")),
    }
    in_specs = tuple(spec_by_name[n] for n in tuple(in_names) + tuple(out_names))
    out_specs = tuple(spec_by_name[n] for n in out_names)
    donate = tuple(range(n_params, n_params + len(out_names)))
    sharded = jax.jit(
        shard_map(
            _body, mesh=mesh, in_specs=in_specs, out_specs=out_specs, check_rep=False
        ),
        donate_argnums=donate,
        keep_unused=True,
    )

    out_sharding = jax.sharding.NamedSharding(mesh, PartitionSpec(("b", "g")))
    zeros_fn = jax.jit(
        lambda: jax.numpy.zeros((N_CORES * S, D), jax.numpy.float16),
        out_shardings=out_sharding,
    )

    _EXEC_CACHE = (sharded, tuple(in_names), mesh, zeros_fn)
    return _EXEC_CACHE


def _rope_tables():
    inv_freq = 1.0 / ROPE_BASE ** (np.arange(0, HD, 2, dtype=np.float32) / HD)
    t = np.arange(S, dtype=np.float32)
    freqs = np.outer(t, inv_freq)  # [S, HD/2]
    return (
        np.ascontiguousarray(np.cos(freqs).T.astype(np.float32)),
        np.ascontiguousarray(np.sin(freqs).T.astype(np.float32)),
    )


def make_global_inputs(x, Wq, Wk, Wv, Wo):
    """Axis-0-concatenated global arrays, keyed by DRAM tensor name."""
    f16 = np.float16
    # per-head evens-then-odds column permutation
    perm_h = np.concatenate([np.arange(0, HD, 2), np.arange(1, HD, 2)])
    perm_q = np.concatenate([h * HD + perm_h for h in range(NH)])
    perm_k = np.concatenate([h * HD + perm_h for h in range(NKV)])
    x = np.asarray(x)
    Wq_p = np.asarray(Wq)[:, perm_q].astype(f16)
    Wk_p = np.asarray(Wk)[:, perm_k].astype(f16)
    Wv = np.asarray(Wv).astype(f16)
    cosT, sinT = _rope_tables()
    csA_g = np.concatenate([cosT, sinT], axis=0).astype(f16)  # [128, S]
    csB_g = np.concatenate([sinT, cosT], axis=0).astype(f16)  # [128, S]
    # [p, sk] additive causal mask for diagonal blocks; exp underflows to 0
    negtri_g = np.where(
        np.arange(128)[None, :] > np.arange(128)[:, None], -60000.0, 0.0
    ).astype(f16)
    irep_g = np.concatenate([np.eye(128)] * G, axis=1).astype(f16)

    xT_g = np.concatenate([x[b].T for b in range(B)], axis=0).astype(f16)
    wq_g = np.concatenate(
        [Wq_p[:, g * G * HD : (g + 1) * G * HD] for g in range(NKV)], axis=0
    )
    wk_g = np.concatenate([Wk_p[:, g * HD : (g + 1) * HD] for g in range(NKV)], axis=0)
    wv_g = np.concatenate([Wv[:, g * HD : (g + 1) * HD] for g in range(NKV)], axis=0)
    wo_g = np.asarray(Wo).astype(f16)  # row-slice concat over g == Wo itself
    return {
        "xT": xT_g,
        "wq": wq_g,
        "wk": wk_g,
        "wv": wv_g,
        "wo": wo_g,
        "csA": csA_g,
        "csB": csB_g,
        "tri": tri,
    }


def run_global(inputs_g, time_exec=False):
    """Run the kernel on pre-built global input arrays; returns [B, S, D] f32."""
    import jax
    import time as _time

    sharded, in_names, mesh, zeros_fn = _get_exec()
    args = [inputs_g[n] for n in in_names]
    out_g = sharded(*args, zeros_fn())
    if time_exec:
        # device_put inputs once, then time execution only
        from jax.sharding import NamedSharding

        dev_args = [
            jax.device_put(a, NamedSharding(mesh, s))
            for a, s in zip(args, sharded_in_specs())
        ]
        jax.block_until_ready(dev_args)
        times = []
        for _ in range(5):
            z = zeros_fn()
            jax.block_until_ready(z)
            t0 = _time.perf_counter()
            o = sharded(*dev_args, z)
            jax.block_until_ready(o)
            times.append(_time.perf_counter() - t0)
        print(f"exec-only times (ms): {[f'{t*1e3:.2f}' for t in times]}")
        out_g = o
    out = (
        np.asarray(out_g)
        .astype(np.float32)
        .reshape(B, NKV, S, D)
        .sum(axis=1, dtype=np.float32)
    )
    return out


def sharded_in_specs():
    from jax.sharding import PartitionSpec

    spec_by_name = {
        "xT": PartitionSpec("b"),
        "wq": PartitionSpec("g"),
        "wk": PartitionSpec("g"),
        "wv": PartitionSpec("g"),
        "wo": PartitionSpec("g"),
        "csA": PartitionSpec(),
        "csB": PartitionSpec(),
        "tri": PartitionSpec(),
    }
    _, in_names, _, _ = _get_exec()
    return [spec_by_name[n] for n in in_names]


def kernel(x, mask, Wq, Wk, Wv, Wo):
    inputs_g = make_global_inputs(x, Wq, Wk, Wv, Wo)
    return run_global(inputs_g)
